# revision 4
# baseline (speedup 1.0000x reference)
"""AttentionBlock Trainium2 kernel: GroupNorm -> QKV -> MHA -> proj -> residual.

Data-parallel over batch B=8 across 8 NeuronCores (one batch image per core).
All matmuls run in bf16 on the TensorEngine (fp32 accumulation in PSUM);
GroupNorm statistics and the residual path stay in fp32.

Per-core layouts (C=512 channels, HW=1024 tokens, 8 heads, hd=64):
  x, xn        [C, HW]   channels on partitions (4 chunks of 128)
  q, k         [C_qk, HW] = qk_sb[128, 8 oc, 1024]; head pair hp lives in
               oc=hp (q) / oc=4+hp (k), heads at partition 0:64 / 64:128
  vT           [HW, C_v] = vt[128, 8 hwc, 8 head, 65] with a ones column
               (65th) so the attention-value matmul also produces the
               softmax denominator.
  scoresT      [k, q] per (head, kchunk): PSUM [128, 1024]; the two heads
               of a pair are issued interleaved at PE row-tiles (0,.) and
               (64,.) so they can execute concurrently in the array.
  E=exp(s*sc)  SBUF bf16 per pair: [128, 2 head, 8 kc, 1024]
  att          [C, HW] bf16 (pair hp -> chunk hp)
  out          [C, HW] fp32 = proj(att) + proj_b + x

PSUM (8 banks): ps_s tag x2 bufs (scores / lead-in qk / proj py0-1) = 4
banks; avA x1 (the in-pair AV accumulator / py2) + avB x1 (rotating:
GN stats, V^T, mid-pair qk, deferred a=0 AV, py3) = 4 banks.  The explicit
avA/avB split keeps the rotating tiles off the long-lived AV slot, which
otherwise serializes every pair boundary.

Softmax denominators: AV psum row 64 -> SBUF -> DRAM -> [128, 8] reshape so
the reciprocal runs spread across all partitions (~0.2us instead of 6.5us
on one), then back through DRAM into a [64, 1024] broadcast for the
normalize multiply.
"""

import sys

if "/opt/trn_rl_repo" not in sys.path:
    sys.path.insert(0, "/opt/trn_rl_repo")

import numpy as np
import ml_dtypes

import concourse.bass as bass
import concourse.tile as tile
from concourse import mybir, bacc
from concourse.bass_utils import run_bass_kernel_spmd

AF = mybir.ActivationFunctionType
ALU = mybir.AluOpType
F32 = mybir.dt.float32
BF16 = mybir.dt.bfloat16

C = 512
HW = 1024
NHEADS = 8
HD = 64
NGROUPS = 32
GSIZE = 16  # channels per group
EPS = 1e-5
SCALE = HD ** -0.5
CC = 4   # channel chunks of 128
OCQK = 8  # q+k output chunks of 128
QC = 2   # 512-wide moving slices per 1024

# If the DVE rejects partition-shifted writes (in partitions 0:64 ->
# out partitions 64:128), flip to False for the stage-then-DMA fallback.
DVE_PARTITION_SHIFT = True


def _build():
    nc = bacc.Bacc("TRN2", target_bir_lowering=False, debug=False, num_devices=8)

    x_d = nc.dram_tensor("x", [C, HW], F32, kind="ExternalInput")
    qw_d = nc.dram_tensor("qw", [C, 3 * C], BF16, kind="ExternalInput")
    pw_d = nc.dram_tensor("pw", [C, C], BF16, kind="ExternalInput")
    qkb_d = nc.dram_tensor("qkb", [128, 8], F32, kind="ExternalInput")
    vbb_d = nc.dram_tensor("vbb", [128, C], F32, kind="ExternalInput")
    pb_d = nc.dram_tensor("pb", [128, 4], F32, kind="ExternalInput")
    gnw_d = nc.dram_tensor("gnw", [128, 4], F32, kind="ExternalInput")
    gnb_d = nc.dram_tensor("gnb", [128, 4], F32, kind="ExternalInput")
    ind_d = nc.dram_tensor("ind", [128, 8], F32, kind="ExternalInput")
    indt_d = nc.dram_tensor("indt", [8, 128], F32, kind="ExternalInput")
    out_d = nc.dram_tensor("out", [C, HW], F32, kind="ExternalOutput")

    with tile.TileContext(nc) as tc:
        with (
            tc.tile_pool(name="consts", bufs=1) as consts,
            tc.tile_pool(name="epool", bufs=2) as epool,
            tc.tile_pool(name="small", bufs=4) as small,
            tc.tile_pool(name="outp", bufs=3) as outp,
            tc.tile_pool(name="drp", bufs=4, space="DRAM") as drp,
            tc.tile_pool(name="ps_s", bufs=2, space="PSUM") as ps_s,
            tc.tile_pool(name="ps_av", bufs=1, space="PSUM") as ps_av,
        ):
            # ---- persistent SBUF tiles + input DMAs ----
            # x split per chunk so GroupNorm starts on chunk 0 after 512KB
            x_sb = consts.tile([128, CC, HW], F32, tag="x")
            x_r = x_d.ap().rearrange("(cc p) hw -> p cc hw", p=128)
            for cc in range(CC):
                nc.sync.dma_start(out=x_sb[:, cc, :], in_=x_r[:, cc, :])
            # small tables next (cheap, gate the GN chain)
            ind = consts.tile([128, 8], F32, tag="ind")
            nc.sync.dma_start(out=ind, in_=ind_d.ap())
            indt = consts.tile([8, 128], F32, tag="indt")
            nc.sync.dma_start(out=indt, in_=indt_d.ap())
            gnw = consts.tile([128, 4], F32, tag="gnw")
            nc.sync.dma_start(out=gnw, in_=gnw_d.ap())
            gnb = consts.tile([128, 4], F32, tag="gnb")
            nc.sync.dma_start(out=gnb, in_=gnb_d.ap())
            qkb = consts.tile([128, 8], F32, tag="qkb")
            nc.sync.dma_start(out=qkb, in_=qkb_d.ap())
            vbb = consts.tile([128, C], F32, tag="vbb")
            nc.sync.dma_start(out=vbb, in_=vbb_d.ap())
            pb = consts.tile([128, 4], F32, tag="pb")
            nc.sync.dma_start(out=pb, in_=pb_d.ap())
            # weights: q/k slice before v slice before proj
            qw_sb = consts.tile([128, CC, 3 * C], BF16, tag="qw")
            qw_r = qw_d.ap().rearrange("(cc p) o -> p cc o", p=128)
            nc.sync.dma_start(out=qw_sb[:, :, 0:2 * C], in_=qw_r[:, :, 0:2 * C])
            nc.sync.dma_start(out=qw_sb[:, :, 2 * C:3 * C], in_=qw_r[:, :, 2 * C:3 * C])
            pw_sb = consts.tile([128, CC, C], BF16, tag="pw")
            nc.sync.dma_start(out=pw_sb, in_=pw_d.ap().rearrange("(cc p) o -> p cc o", p=128))

            xn_sb = consts.tile([128, CC, HW], BF16, tag="xn")
            qk_sb = consts.tile([128, OCQK, HW], BF16, tag="qk")
            vt_sb = consts.tile([128, 8, NHEADS, HD + 1], BF16, tag="vt")
            att_t = [consts.tile([128, HW], BF16, tag=f"att{i}", name=f"att{i}") for i in range(CC)]

            # ones column of vT (softmax denominator trick)
            nc.vector.memset(vt_sb[:, :, :, HD:HD + 1], 1.0)

            # ---- GroupNorm, fully per-chunk so xn[cc] unblocks as x[cc] lands ----
            for cc in range(CC):
                st = small.tile([128, 2, 6], F32, tag="gn_st", name=f"gn_st{cc}")
                nc.vector.bn_stats(out=st[:, 0, :], in_=x_sb[:, cc, 0:512])
                nc.vector.bn_stats(out=st[:, 1, :], in_=x_sb[:, cc, 512:1024])
                mv = small.tile([128, 2], F32, tag="gn_mv", name=f"gn_mv{cc}")
                nc.vector.bn_aggr(out=mv, in_=st)
                # mv col1 <- E[x^2]_c = var_c + mean_c^2 (in place)
                scr = small.tile([128, 1], F32, tag="gn_scr", name=f"gn_scr{cc}")
                nc.vector.tensor_mul(out=scr, in0=mv[:, 0:1], in1=mv[:, 0:1])
                nc.vector.tensor_add(out=mv[:, 1:2], in0=mv[:, 1:2], in1=scr)
                # group means over 16-channel blocks (ind carries the 1/16): [8, 2]
                pg = ps_av.tile([8, 2], F32, tag="avB", bufs=1, name=f"gn_pg{cc}")
                nc.tensor.matmul(out=pg, lhsT=ind, rhs=mv, start=True, stop=True)
                # sg cols: [mean_g, ex2->rstd_g, vpe, t, r] (PSUM copied to SBUF
                # first: 8-partition scalar_tensor_tensor can't read PSUM)
                sg = small.tile([8, 5], F32, tag="gn_sg", name=f"gn_sg{cc}")
                nc.vector.tensor_copy(out=sg[:, 0:2], in_=pg)
                nc.vector.scalar_tensor_tensor(out=sg[:, 2:3], in0=sg[:, 0:1], scalar=-1.0, in1=sg[:, 0:1], op0=ALU.mult, op1=ALU.mult)
                nc.vector.scalar_tensor_tensor(out=sg[:, 2:3], in0=sg[:, 1:2], scalar=EPS, in1=sg[:, 2:3], op0=ALU.add, op1=ALU.add)
                # rstd = 1/sqrt(vpe) with one Newton polish (ACT sqrt is low-precision)
                nc.scalar.activation(out=sg[:, 3:4], in_=sg[:, 2:3], func=AF.Sqrt, bias=0.0, scale=1.0)
                nc.vector.reciprocal(out=sg[:, 4:5], in_=sg[:, 3:4])
                nc.vector.scalar_tensor_tensor(out=sg[:, 3:4], in0=sg[:, 4:5], scalar=1.0, in1=sg[:, 4:5], op0=ALU.mult, op1=ALU.mult)
                nc.vector.scalar_tensor_tensor(out=sg[:, 3:4], in0=sg[:, 3:4], scalar=-0.5, in1=sg[:, 2:3], op0=ALU.mult, op1=ALU.mult)
                nc.vector.scalar_tensor_tensor(out=sg[:, 1:2], in0=sg[:, 3:4], scalar=1.5, in1=sg[:, 4:5], op0=ALU.add, op1=ALU.mult)
                # broadcast [mean_g, rstd_g] to channels: [128, 2] = indt.T @ sg
                pbc = ps_av.tile([128, 2], F32, tag="avB", bufs=1, name=f"gn_pbc{cc}")
                nc.tensor.matmul(out=pbc, lhsT=indt, rhs=sg[:, 0:2], start=True, stop=True)
                # A = rstd_bc * gnw ; B = gnb - mean_bc * A
                ab = small.tile([128, 2], F32, tag="gn_ab", name=f"gn_ab{cc}")
                nc.vector.tensor_mul(out=ab[:, 0:1], in0=pbc[:, 1:2], in1=gnw[:, cc:cc + 1])
                nc.vector.scalar_tensor_tensor(out=ab[:, 1:2], in0=pbc[:, 0:1], scalar=-1.0, in1=ab[:, 0:1], op0=ALU.mult, op1=ALU.mult)
                nc.vector.tensor_add(out=ab[:, 1:2], in0=gnb[:, cc:cc + 1], in1=ab[:, 1:2])
                nc.vector.tensor_scalar(out=xn_sb[:, cc, :], in0=x_sb[:, cc, :], scalar1=ab[:, 0:1], scalar2=ab[:, 1:2], op0=ALU.mult, op1=ALU.add)

            # ---- q/k production (lead-ins on ps_s; mid-pair ones on avB) ----
            def make_qk(oc, pool, tag):
                pq = pool.tile([128, HW], F32, tag=tag, bufs=None if tag == "ps_s" else 1, name=f"pq{oc}")
                for cc in range(CC):
                    for q2 in range(QC):
                        nc.tensor.matmul(
                            out=pq[:, q2 * 512:(q2 + 1) * 512],
                            lhsT=qw_sb[:, cc, oc * 128:(oc + 1) * 128],
                            rhs=xn_sb[:, cc, q2 * 512:(q2 + 1) * 512],
                            start=(cc == 0), stop=(cc == CC - 1),
                        )
                nc.vector.tensor_scalar_add(out=qk_sb[:, oc, :], in0=pq[:], scalar1=qkb[:, oc:oc + 1])

            make_qk(0, ps_s, "ps_s")
            make_qk(4, ps_s, "ps_s")

            # ---- V^T chunks (2 hw-chunks per psum tile), woven into pair 0 ----
            def vt_chunk(i):
                pv = ps_av.tile([128, 2, 512], F32, tag="avB", bufs=1, name=f"pv{i}")
                for h2 in range(2):
                    hwc = 2 * i + h2
                    for cc in range(CC):
                        nc.tensor.matmul(
                            out=pv[:, h2, :],
                            lhsT=xn_sb[:, cc, hwc * 128:(hwc + 1) * 128],
                            rhs=qw_sb[:, cc, 2 * C:3 * C],
                            start=(cc == 0), stop=(cc == CC - 1),
                        )
                for h2 in range(2):
                    hwc = 2 * i + h2
                    nc.vector.tensor_add(
                        out=vt_sb[:, hwc, :, 0:HD],
                        in0=pv[:, h2, :].rearrange("p (h d) -> p h d", d=HD),
                        in1=vbb[:].rearrange("p (h d) -> p h d", d=HD),
                    )

            # ---- attention ----
            def normalize_head(hp, a, av_tile):
                # copy AV block PSUM->SBUF (releases the PSUM slot)
                avs = small.tile([65, HW], F32, tag="avs", name=f"avs{hp}_{a}")
                nc.vector.tensor_copy(out=avs, in_=av_tile[:, :])
                # denominator row -> DRAM -> [128, 8] so the reciprocal runs
                # across all partitions, then back -> DRAM -> broadcast
                dd = drp.tile([HW], F32, tag="dd", name=f"dd{hp}_{a}")
                nc.sync.dma_start(out=dd, in_=avs[64:65, :])
                dt = small.tile([128, 8], F32, tag="dt", name=f"dt{hp}_{a}")
                nc.sync.dma_start(
                    out=dt,
                    in_=bass.AP(tensor=dd.tensor, offset=dd.offset, ap=[[8, 128], [1, 8]]),
                )
                nc.vector.reciprocal(out=dt, in_=dt)
                rr = drp.tile([HW], F32, tag="rr", name=f"rr{hp}_{a}")
                nc.sync.dma_start(out=rr, in_=dt)
                sbc = small.tile([64, HW], F32, tag="sbc", name=f"sbc{hp}_{a}")
                nc.sync.dma_start(
                    out=sbc,
                    in_=bass.AP(tensor=rr.tensor, offset=rr.offset, ap=[[0, 64]] + list(rr.ap)),
                )
                if a == 0:
                    nc.vector.tensor_mul(out=att_t[hp][0:64, :], in0=avs[0:64, :], in1=sbc)
                elif DVE_PARTITION_SHIFT:
                    nc.vector.tensor_mul(out=att_t[hp][64:128, :], in0=avs[0:64, :], in1=sbc)
                else:
                    sc = small.tile([64, HW], BF16, tag="att_sc", name=f"attsc{hp}_{a}")
                    nc.vector.tensor_mul(out=sc, in0=avs[0:64, :], in1=sbc)
                    nc.sync.dma_start(out=att_t[hp][64:128, :], in_=sc)

            E_prev = None
            av0_prev = None

            for hp in range(4):
                E = epool.tile([128, 2, 8, HW], BF16, tag="E", name=f"E{hp}")
                av1 = ps_av.tile([65, HW], F32, tag="avA", bufs=1, name=f"av1_{hp}")
                av0_cur = None
                if hp == 3:
                    av0_cur = ps_av.tile([65, HW], F32, tag="avB", bufs=1, name="av0_3")
                for kc in range(8):
                    if hp == 0 and kc % 2 == 0:
                        vt_chunk(kc // 2)
                    psA = ps_s.tile([128, HW], F32, tag="ps_s", name=f"s{hp}_{kc}_1")
                    psB = ps_s.tile([128, HW], F32, tag="ps_s", name=f"s{hp}_{kc}_0")
                    # interleave the two heads at PE row-tiles (64,.) / (0,.)
                    for q2 in range(QC):
                        w = slice(q2 * 512, (q2 + 1) * 512)
                        nc.tensor.matmul(
                            out=psA[:, w],
                            lhsT=qk_sb[64:128, 4 + hp, kc * 128:(kc + 1) * 128],
                            rhs=qk_sb[64:128, hp, w],
                            start=True, stop=True,
                        )
                        nc.tensor.matmul(
                            out=psB[:, w],
                            lhsT=qk_sb[0:64, 4 + hp, kc * 128:(kc + 1) * 128],
                            rhs=qk_sb[0:64, hp, w],
                            start=True, stop=True,
                        )
                    nc.scalar.activation(out=E[:, 1, kc, :], in_=psA[:], func=AF.Exp, scale=SCALE)
                    nc.scalar.activation(out=E[:, 0, kc, :], in_=psB[:], func=AF.Exp, scale=SCALE)
                    # head a=1 AV streams within the pair
                    for q2 in range(QC):
                        w = slice(q2 * 512, (q2 + 1) * 512)
                        nc.tensor.matmul(
                            out=av1[:, w],
                            lhsT=vt_sb[:, kc, 2 * hp + 1, :],
                            rhs=E[:, 1, kc, w],
                            start=(kc == 0), stop=(kc == 7),
                        )
                    if hp < 3:
                        # previous pair's a=0 AV, 2 kc-chunks per slot (kc 1..4)
                        if E_prev is not None and 1 <= kc <= 4:
                            for bkc in (2 * (kc - 1), 2 * (kc - 1) + 1):
                                for q2 in range(QC):
                                    w = slice(q2 * 512, (q2 + 1) * 512)
                                    nc.tensor.matmul(
                                        out=av0_prev[:, w],
                                        lhsT=vt_sb[:, bkc, 2 * (hp - 1), :],
                                        rhs=E_prev[:, 0, bkc, w],
                                        start=(bkc == 0), stop=(bkc == 7),
                                    )
                            if kc == 4:
                                normalize_head(hp - 1, 0, av0_prev)
                        if kc == 4:
                            make_qk(hp + 1, ps_av, "avB")
                        if kc == 6:
                            make_qk(4 + hp + 1, ps_av, "avB")
                    else:
                        # last pair: finish pair 2's a=0 early, stream own a=0
                        if kc in (0, 1):
                            for bkc in range(4 * kc, 4 * kc + 4):
                                for q2 in range(QC):
                                    w = slice(q2 * 512, (q2 + 1) * 512)
                                    nc.tensor.matmul(
                                        out=av0_prev[:, w],
                                        lhsT=vt_sb[:, bkc, 4, :],
                                        rhs=E_prev[:, 0, bkc, w],
                                        start=(bkc == 0), stop=(bkc == 7),
                                    )
                            if kc == 1:
                                normalize_head(2, 0, av0_prev)
                        if kc >= 3:
                            for q2 in range(QC):
                                w = slice(q2 * 512, (q2 + 1) * 512)
                                nc.tensor.matmul(
                                    out=av0_cur[:, w],
                                    lhsT=vt_sb[:, kc, 6, :],
                                    rhs=E[:, 0, kc, w],
                                    start=(kc == 3), stop=False,
                                )
                if hp == 3:
                    # catch-up: kc 0..2 of the last pair's a=0 head
                    for kc in range(3):
                        for q2 in range(QC):
                            w = slice(q2 * 512, (q2 + 1) * 512)
                            nc.tensor.matmul(
                                out=av0_cur[:, w],
                                lhsT=vt_sb[:, kc, 6, :],
                                rhs=E[:, 0, kc, w],
                                start=False, stop=(kc == 2),
                            )
                normalize_head(hp, 1, av1)
                if hp == 3:
                    normalize_head(3, 0, av0_cur)
                E_prev = E
                if hp < 3:
                    av0_prev = ps_av.tile([65, HW], F32, tag="avB", bufs=1, name=f"av0_{hp}")

            # ---- proj + residual tail ----
            def proj_mm(py, oc, cc):
                for q2 in range(QC):
                    nc.tensor.matmul(
                        out=py[:, q2 * 512:(q2 + 1) * 512],
                        lhsT=pw_sb[:, cc, oc * 128:(oc + 1) * 128],
                        rhs=att_t[cc][:, q2 * 512:(q2 + 1) * 512],
                        start=(cc == 0), stop=(cc == CC - 1),
                    )

            def proj_epilogue(py, oc):
                ot = outp.tile([128, HW], F32, tag="ot", name=f"ot{oc}")
                nc.vector.scalar_tensor_tensor(out=ot, in0=py[:], scalar=pb[:, oc:oc + 1], in1=x_sb[:, oc, :], op0=ALU.add, op1=ALU.add)
                nc.sync.dma_start(out=out_d.ap()[oc * 128:(oc + 1) * 128, :], in_=ot)

            # att chunks 0..2 contract while att3's normalize chain drains;
            # only the cc=3 matmuls wait on the last attention chunk.
            pys = []
            for oc, (pool, tag) in enumerate(
                ((ps_s, "ps_s"), (ps_s, "ps_s"), (ps_av, "avA"), (ps_av, "avB"))
            ):
                py = pool.tile([128, HW], F32, tag=tag, bufs=None if tag == "ps_s" else 1, name=f"py{oc}")
                pys.append(py)
                for cc in range(CC - 1):
                    proj_mm(py, oc, cc)
            for oc in range(4):
                proj_mm(pys[oc], oc, CC - 1)
                proj_epilogue(pys[oc], oc)

    nc.compile()
    return nc


_NC_CACHE = None


def _get_nc():
    global _NC_CACHE
    if _NC_CACHE is None:
        _NC_CACHE = _build()
    return _NC_CACHE


def _prep_in_maps(inputs):
    x = np.asarray(inputs["x"], np.float32)
    gn_w = np.asarray(inputs["gn_w"], np.float32)
    gn_b = np.asarray(inputs["gn_b"], np.float32)
    qkv_w = np.asarray(inputs["qkv_w"], np.float32)
    qkv_b = np.asarray(inputs["qkv_b"], np.float32)
    proj_w = np.asarray(inputs["proj_w"], np.float32)
    proj_b = np.asarray(inputs["proj_b"], np.float32)

    B = x.shape[0]
    xr = x.reshape(B, C, HW)
    qwT = np.ascontiguousarray(qkv_w.T).astype(ml_dtypes.bfloat16)
    pwT = np.ascontiguousarray(proj_w.T).astype(ml_dtypes.bfloat16)
    qkb = np.ascontiguousarray(qkv_b[: 2 * C].reshape(8, 128).T)
    vbb = np.ascontiguousarray(np.broadcast_to(qkv_b[2 * C:], (128, C)))
    pb = np.ascontiguousarray(proj_b.reshape(4, 128).T)
    gnw = np.ascontiguousarray(gn_w.reshape(4, 128).T)
    gnb = np.ascontiguousarray(gn_b.reshape(4, 128).T)
    indm = np.zeros((128, 8), np.float32)
    indm[np.arange(128), np.arange(128) // GSIZE] = 1.0 / GSIZE
    ind01 = np.zeros((128, 8), np.float32)
    ind01[np.arange(128), np.arange(128) // GSIZE] = 1.0
    indt = np.ascontiguousarray(ind01.T)
    shared = dict(qw=qwT, pw=pwT, qkb=qkb, vbb=vbb, pb=pb, gnw=gnw, gnb=gnb, ind=indm, indt=indt)
    return [dict(x=np.ascontiguousarray(xr[b]), **shared) for b in range(B)]


def kernel(**inputs):
    nc = _get_nc()
    in_maps = _prep_in_maps(inputs)
    res = run_bass_kernel_spmd(nc, in_maps, core_ids=list(range(8)))
    out = np.stack([r["out"] for r in res.results])
    return out.reshape(8, C, 32, 32).astype(np.float32)


def run_profiled(inputs):
    """kernel() + NTFF profiling; returns (output, exec_time_ns, trace_path)."""
    import types

    import antenv

    if "antenv.axon_hooks" not in sys.modules:
        hooks_mod = types.ModuleType("antenv.axon_hooks")
        _hook = [None]
        hooks_mod.set_axon_ntff_profile_hook = lambda h: _hook.__setitem__(0, h)
        hooks_mod.get_axon_ntff_profile_hook = lambda: _hook[0]
        sys.modules["antenv.axon_hooks"] = hooks_mod
        antenv.axon_hooks = hooks_mod
        from trn_agent_boot.trn_boot import _ntff_profile_via_ctypes

        hooks_mod.set_axon_ntff_profile_hook(_ntff_profile_via_ctypes("/opt/axon/libaxon_pjrt.so"))

    nc = _get_nc()
    in_maps = _prep_in_maps(inputs)
    res = run_bass_kernel_spmd(nc, in_maps, core_ids=list(range(8)), trace=True)
    out = np.stack([r["out"] for r in res.results]).reshape(8, C, 32, 32).astype(np.float32)
    trace = res.instructions_and_trace[1] if res.instructions_and_trace else None
    return out, res.exec_time_ns, trace
